# revision 11
# baseline (speedup 1.0000x reference)
"""Multihead causal attention on 8 TRN2 NeuronCores.

Sharding: core = (batch b, head-group hg): b = core//2, hg = core%2.
Each core gets x[b] (full sequence, [2048, 1024]) plus the weight rows for
its 8 heads (W[hg*512:(hg+1)*512, :]), computes Q/K/V projections and
causal attention for those (batch, head) pairs, and writes Y transposed
as [8, 64, 2048] (head, dh, seq); the host transposes back on gather.

On-device dataflow (per core):
  - x.T and W.T tiles built with PE transposes (contraction dim must sit
    on SBUF partitions).
  - Q.T, K.T in [d, s] layout, V in [s, d] layout; matmuls run as
    float32r (1 cyc/row vs 4 for strict fp32).
  - Attention in transposed-score layout: scoresT[k, q] = K @ Q.T per
    head, two heads packed in the 128-row PE array (K=64 each).
  - Softmax without a max pass (scaled scores are O(4)); exp on ScalarE
    (PSUM -> bf16 SBUF), causal mask via zero-prefix memset + one
    [128,128] triangular 0/1 multiply per diagonal tile.
  - PV matmul in bf16 with a ones-column appended to V: out [65, 512]
    rows 0..63 = unnormalized Y.T, row 64 = softmax denominator.
  - Normalize: reciprocal of the denominator row, partition-broadcast
    via a DRAM bounce, one DVE multiply.
"""
import numpy as np
import ml_dtypes

import concourse.bass as bass
import concourse.tile as tile
from concourse import bacc, mybir
from concourse.bass_utils import run_bass_kernel_spmd

F32 = mybir.dt.float32
F32R = mybir.dt.float32r
BF16 = mybir.dt.bfloat16
EXP = mybir.ActivationFunctionType.Exp

B, S, D, H, DH = 4, 2048, 1024, 16, 64
N_CORES = 8
H_LOC = 8          # heads per core
D_LOC = H_LOC * DH  # 512: projection output dim per core
N_CT = D // 128     # 8 contraction tiles
N_ST = S // 128     # 16 sequence tiles of 128
N_QT = S // 512     # 4 q-chunks of 512
SCALE = 1.0 / np.sqrt(DH)

_NC_CACHE = {}


def build_nc():
    nc = bacc.Bacc("TRN2", target_bir_lowering=False, debug=False,
                   num_devices=N_CORES)
    x = nc.dram_tensor("x", [S, D], F32, kind="ExternalInput").ap()
    wq = nc.dram_tensor("wq", [D_LOC, D], F32, kind="ExternalInput").ap()
    wk = nc.dram_tensor("wk", [D_LOC, D], F32, kind="ExternalInput").ap()
    wv = nc.dram_tensor("wv", [D_LOC, D], F32, kind="ExternalInput").ap()
    out = nc.dram_tensor("out", [H_LOC, DH, S], F32, kind="ExternalOutput").ap()

    ident_dram = nc.inline_tensor(np.eye(128, dtype=np.float32), name="ident")
    # tri[kk, qq] = 1 iff qq >= kk (valid: query position >= key position)
    tri_np = (np.arange(128)[None, :] >= np.arange(128)[:, None])
    tri_dram = nc.inline_tensor(tri_np.astype(ml_dtypes.bfloat16), name="tri")

    with tile.TileContext(nc) as tc:
        with tc.tile_pool(name="consts", bufs=1) as consts, \
             tc.tile_pool(name="pers", bufs=1) as pers:
            ident = consts.tile([128, 128], F32)
            nc.sync.dma_start(ident[:], ident_dram.ap())
            tri = consts.tile([128, 128], BF16)
            nc.sync.dma_start(tri[:], tri_dram.ap())

            # persistent per-core tensors
            QT = [pers.tile([128, S], BF16, tag=f"QT{i}", name=f"QT{i}") for i in range(4)]
            KT = [pers.tile([128, S], BF16, tag=f"KT{i}", name=f"KT{i}") for i in range(4)]
            VP = [pers.tile([128, H_LOC, DH + 1], BF16, tag=f"VP{i}", name=f"VP{i}")
                  for i in range(N_ST)]

            # ---- phase A+B: transposes + projections ----
            with tc.tile_pool(name="stage", bufs=3) as stage, \
                 tc.tile_pool(name="xT", bufs=1) as xT_pool, \
                 tc.tile_pool(name="WT", bufs=1) as WT_pool, \
                 tc.tile_pool(name="psT", bufs=6, space="PSUM") as psT, \
                 tc.tile_pool(name="psP", bufs=2, space="PSUM") as psP:

                # x.T: xT[ct][cc, s] = x[s, ct*128+cc]
                xT = [xT_pool.tile([128, S], F32R, tag=f"xT{i}", name=f"xT{i}")
                      for i in range(N_CT)]
                for st in range(N_ST):
                    xs = stage.tile([128, D], F32, tag="xs")
                    nc.sync.dma_start(xs[:], x[st * 128:(st + 1) * 128, :])
                    for ct in range(N_CT):
                        pt = psT.tile([128, 128], F32, tag="pt")
                        nc.tensor.transpose(
                            pt[:], xs[:, ct * 128:(ct + 1) * 128], ident[:])
                        nc.scalar.copy(
                            xT[ct][:, st * 128:(st + 1) * 128], pt[:])

                # W.T per weight; Q/K projections into [d, s] layout,
                # V projection into [s, d] layout with ones column.
                WT = [WT_pool.tile([128, D_LOC], F32R, tag=f"WT{i}", name=f"WT{i}")
                      for i in range(N_CT)]
                for wi, w in ((2, wv), (1, wk), (0, wq)):
                    for dt in range(4):
                        ws = stage.tile([128, D], F32, tag="ws")
                        nc.sync.dma_start(
                            ws[:], w[dt * 128:(dt + 1) * 128, :])
                        for ct in range(N_CT):
                            pt = psT.tile([128, 128], F32, tag="pt")
                            nc.tensor.transpose(
                                pt[:], ws[:, ct * 128:(ct + 1) * 128],
                                ident[:])
                            nc.vector.tensor_copy(
                                WT[ct][:, dt * 128:(dt + 1) * 128], pt[:])
                    if wi < 2:  # Q or K: out[dloc, s]
                        dst = QT if wi == 0 else KT
                        for dt in range(4):
                            for qc in range(N_QT):
                                pp = psP.tile([128, 512], F32, tag="pp")
                                for ct in range(N_CT):
                                    nc.tensor.matmul(
                                        pp[:],
                                        WT[ct][:, dt * 128:(dt + 1) * 128],
                                        xT[ct][:, qc * 512:(qc + 1) * 512],
                                        start=(ct == 0), stop=(ct == N_CT - 1))
                                nc.vector.tensor_copy(
                                    dst[dt][:, qc * 512:(qc + 1) * 512], pp[:])
                    else:  # V: out[s, dloc] -> VP interleaved by head
                        for st in range(N_ST):
                            pp = psP.tile([128, 512], F32, tag="pp")
                            for ct in range(N_CT):
                                nc.tensor.matmul(
                                    pp[:],
                                    xT[ct][:, st * 128:(st + 1) * 128],
                                    WT[ct][:],
                                    start=(ct == 0), stop=(ct == N_CT - 1))
                            nc.vector.tensor_copy(
                                VP[st][:, :, 0:DH],
                                pp[:].rearrange("p (h d) -> p h d", h=H_LOC))
                            nc.vector.memset(VP[st][:, :, DH:DH + 1], 1.0)

            # ---- phase C: attention ----
            with tc.tile_pool(name="epool", bufs=6) as epool, \
                 tc.tile_pool(name="norm", bufs=4) as norm, \
                 tc.tile_pool(name="psS", bufs=3, space="PSUM") as psS, \
                 tc.tile_pool(name="psY", bufs=1, space="PSUM") as psY:
                for g in range(4):          # head pair: local heads 2g, 2g+1
                    for qt in range(N_QT):  # q-chunk of 512
                        n_kt = 4 * (qt + 1)
                        q0, q1 = qt * 512, (qt + 1) * 512
                        yy = [psY.tile([DH + 1, 512], F32, tag=f"y{hh}", name=f"y{hh}")
                              for hh in range(2)]
                        for kp in range(n_kt // 2):
                            kts = (2 * kp, 2 * kp + 1)
                            ps2 = [psS.tile([128, 1024], F32, tag="s", name="s")
                                   for _ in range(2)]
                            # scores: group by head so consecutive matmuls
                            # stay on one PSUM tile (no bank cycling)
                            for hh in range(2):
                                rows = slice(hh * 64, hh * 64 + 64)
                                for j, kt in enumerate(kts):
                                    k0, k1 = kt * 128, (kt + 1) * 128
                                    nc.tensor.matmul(
                                        ps2[hh][:, j * 512:(j + 1) * 512],
                                        KT[g][rows, k0:k1],
                                        QT[g][rows, q0:q1],
                                        start=True, stop=True)
                            ee = [epool.tile([128, 1024], BF16, tag="e", name="e")
                                  for _ in range(2)]
                            diag = (kts[0] >= 4 * qt)
                            for hh in range(2):
                                if not diag:
                                    nc.scalar.activation(
                                        ee[hh][:], ps2[hh][:], EXP,
                                        scale=SCALE)
                                else:
                                    for j, kt in enumerate(kts):
                                        off = kt * 128 - qt * 512
                                        cb = j * 512
                                        if off > 0:
                                            nc.gpsimd.memset(
                                                ee[hh][:, cb:cb + off], 0.0)
                                        nc.scalar.activation(
                                            ee[hh][:, cb + off:cb + 512],
                                            ps2[hh][:, cb + off:cb + 512],
                                            EXP, scale=SCALE)
                                        nc.vector.tensor_mul(
                                            ee[hh][:, cb + off:cb + off + 128],
                                            ee[hh][:, cb + off:cb + off + 128],
                                            tri[:])
                            for hh in range(2):
                                for j, kt in enumerate(kts):
                                    nc.tensor.matmul(
                                        yy[hh][:],
                                        VP[kt][:, 2 * g + hh, :],
                                        ee[hh][:, j * 512:(j + 1) * 512],
                                        start=(kt == 0), stop=(kt == n_kt - 1))
                        for hh in range(2):
                            den = norm.tile([1, 512], F32, tag="den")
                            nc.vector.tensor_copy(den[:], yy[hh][DH:DH + 1, :])
                            rd = norm.tile([1, 512], F32, tag="rd")
                            nc.vector.reciprocal_approx_fast(rd[:], den[:])
                            rdb = norm.tile([DH, 512], F32, tag="rdb")
                            nc.gpsimd.partition_broadcast(rdb[:], rd[:])
                            yn = norm.tile([DH, 512], F32, tag="yn")
                            nc.vector.tensor_mul(yn[:], yy[hh][0:DH, :], rdb[:])
                            nc.sync.dma_start(
                                out[2 * g + hh, :, q0:q1], yn[:])
    nc.compile()
    return nc


def get_nc():
    if "nc" not in _NC_CACHE:
        _NC_CACHE["nc"] = build_nc()
    return _NC_CACHE["nc"]


def make_in_maps(x, W_q, W_k, W_v):
    in_maps = []
    for core in range(N_CORES):
        b, hg = core // 2, core % 2
        rows = slice(hg * D_LOC, (hg + 1) * D_LOC)
        in_maps.append({
            "x": np.ascontiguousarray(x[b], dtype=np.float32),
            "wq": np.ascontiguousarray(W_q[rows], dtype=np.float32),
            "wk": np.ascontiguousarray(W_k[rows], dtype=np.float32),
            "wv": np.ascontiguousarray(W_v[rows], dtype=np.float32),
        })
    return in_maps


def assemble(results):
    Y = np.empty((B, H, S, DH), dtype=np.float32)
    for core in range(N_CORES):
        b, hg = core // 2, core % 2
        yc = results[core]["out"]  # [H_LOC, DH, S]
        Y[b, hg * H_LOC:(hg + 1) * H_LOC] = yc.transpose(0, 2, 1)
    return Y


def kernel(x, W_q, W_k, W_v):
    nc = get_nc()
    in_maps = make_in_maps(x, W_q, W_k, W_v)
    res = run_bass_kernel_spmd(nc, in_maps, list(range(N_CORES)))
    return assemble(res.results)


# revision 20
# speedup vs baseline: 1.0246x; 1.0246x over previous
"""Multihead causal attention on 8 TRN2 NeuronCores.

Sharding: core = (batch b, head-group hg): b = core//2, hg = core%2.
Each core gets x[b] (full sequence, [2048, 1024]) plus the weight rows for
its 8 heads (W[hg*512:(hg+1)*512, :]), computes Q/K/V projections and
causal attention for those (batch, head) pairs, and writes Y transposed
as [8, 64, 2048] (head, dh, seq); the host transposes back on gather.

On-device dataflow (per core):
  - x.T and W.T tiles built with PE transposes (contraction dim must sit
    on SBUF partitions).
  - Q.T, K.T in [d, s] layout, V in [s, d] layout; matmuls run as
    float32r (1 cyc/row vs 4 for strict fp32).
  - Attention in transposed-score layout: scoresT[k, q] = K @ Q.T per
    head, two heads packed in the 128-row PE array (K=64 each).
  - Softmax without a max pass (scaled scores are O(4)); exp on ScalarE
    (PSUM -> bf16 SBUF), causal mask via zero-prefix memset + one
    [128,128] triangular 0/1 multiply per diagonal tile.
  - PV matmul in bf16 with a ones-column appended to V: out [65, 512]
    rows 0..63 = unnormalized Y.T, row 64 = softmax denominator.
  - Normalize: reciprocal of the denominator row, partition-broadcast
    via a DRAM bounce, one DVE multiply.
"""
import numpy as np
import ml_dtypes

import concourse.bass as bass
import concourse.tile as tile
from concourse import bacc, mybir
from concourse.bass_utils import run_bass_kernel_spmd

F32 = mybir.dt.float32
F32R = mybir.dt.float32r
BF16 = mybir.dt.bfloat16
EXP = mybir.ActivationFunctionType.Exp

B, S, D, H, DH = 4, 2048, 1024, 16, 64
N_CORES = 8
H_LOC = 8          # heads per core
D_LOC = H_LOC * DH  # 512: projection output dim per core
N_CT = D // 128     # 8 contraction tiles
N_ST = S // 128     # 16 sequence tiles of 128
N_QT = S // 512     # 4 q-chunks of 512
SCALE = 1.0 / np.sqrt(DH)

_NC_CACHE = {}


def build_nc():
    nc = bacc.Bacc("TRN2", target_bir_lowering=False, debug=False,
                   num_devices=N_CORES)
    xb = nc.dram_tensor("xb", [S, D], BF16, kind="ExternalInput").ap()
    wqb = nc.dram_tensor("wqb", [D_LOC, D], BF16, kind="ExternalInput").ap()
    wkb = nc.dram_tensor("wkb", [D_LOC, D], BF16, kind="ExternalInput").ap()
    wvb = nc.dram_tensor("wvb", [D_LOC, D], BF16, kind="ExternalInput").ap()
    out = nc.dram_tensor("out", [H_LOC, DH, S], F32, kind="ExternalOutput").ap()

    # tri[kk, qq] = 1 iff qq >= kk (valid: query position >= key position)
    tri_np = (np.arange(128)[None, :] >= np.arange(128)[:, None])
    tri_dram = nc.inline_tensor(tri_np.astype(ml_dtypes.bfloat16), name="tri")

    with tile.TileContext(nc) as tc:
        with tc.tile_pool(name="consts", bufs=1) as consts, \
             tc.tile_pool(name="pers", bufs=1) as pers:
            tri = consts.tile([128, 128], BF16)
            nc.sync.dma_start(tri[:], tri_dram.ap())

            # persistent per-core tensors
            QT = [pers.tile([128, S], BF16, tag=f"QT{i}", name=f"QT{i}") for i in range(4)]
            KT = [pers.tile([128, S], BF16, tag=f"KT{i}", name=f"KT{i}") for i in range(4)]
            VP = [pers.tile([128, H_LOC, DH + 1], BF16, tag=f"VP{i}", name=f"VP{i}")
                  for i in range(N_ST)]

            # ---- phase A+B: DMA-transposes + projections (all bf16) ----
            with tc.tile_pool(name="xT", bufs=1) as xT_pool, \
                 tc.tile_pool(name="WT", bufs=1) as WT_pool, \
                 tc.tile_pool(name="psP", bufs=2, space="PSUM") as psP:

                # x.T via hardware DMA transpose: xT[ct][cc, s] = x[s, ct*128+cc]
                xT = [xT_pool.tile([128, S], BF16, tag=f"xT{i}", name=f"xT{i}")
                      for i in range(N_CT)]
                for ct in range(N_CT):
                    nc.sync.dma_start_transpose(
                        xT[ct][:], xb[:, ct * 128:(ct + 1) * 128])

                WT = [WT_pool.tile([128, D_LOC], BF16, tag=f"WT{i}",
                                   name=f"WT{i}") for i in range(N_CT)]
                for wi, w in ((2, wvb), (1, wkb), (0, wqb)):
                    for ct in range(N_CT):
                        nc.sync.dma_start_transpose(
                            WT[ct][:], w[:, ct * 128:(ct + 1) * 128])
                    if wi < 2:  # Q or K: out[dloc, s]
                        dst = QT if wi == 0 else KT
                        for dt in range(4):
                            for qc in range(N_QT):
                                pp = psP.tile([128, 512], F32, tag="pp",
                                              name="pp")
                                for ct in range(N_CT):
                                    nc.tensor.matmul(
                                        pp[:],
                                        WT[ct][:, dt * 128:(dt + 1) * 128],
                                        xT[ct][:, qc * 512:(qc + 1) * 512],
                                        start=(ct == 0), stop=(ct == N_CT - 1))
                                nc.vector.tensor_copy(
                                    dst[dt][:, qc * 512:(qc + 1) * 512], pp[:])
                    else:  # V: out[s, dloc] -> VP interleaved by head
                        for st in range(N_ST):
                            pp = psP.tile([128, 512], F32, tag="pp", name="pp")
                            for ct in range(N_CT):
                                nc.tensor.matmul(
                                    pp[:],
                                    xT[ct][:, st * 128:(st + 1) * 128],
                                    WT[ct][:],
                                    start=(ct == 0), stop=(ct == N_CT - 1))
                            nc.vector.tensor_copy(
                                VP[st][:, :, 0:DH],
                                pp[:].rearrange("p (h d) -> p h d", h=H_LOC))
                            nc.vector.memset(VP[st][:, :, DH:DH + 1], 1.0)

            # ---- phase C: attention ----
            with tc.tile_pool(name="epool", bufs=6) as epool, \
                 tc.tile_pool(name="norm", bufs=4) as norm, \
                 tc.tile_pool(name="psS", bufs=2, space="PSUM") as psS, \
                 tc.tile_pool(name="psY", bufs=2, space="PSUM") as psY:
                for g in range(4):          # head pair: local heads 2g, 2g+1
                    for qt in range(N_QT):  # q-chunk of 512
                        n_kt = 4 * (qt + 1)
                        q0, q1 = qt * 512, (qt + 1) * 512
                        yy = [psY.tile([DH + 1, 512], F32, tag=f"y{hh}", name=f"y{hh}")
                              for hh in range(2)]
                        for kp in range(n_kt // 2):
                            kts = (2 * kp, 2 * kp + 1)
                            ps2 = [psS.tile([128, 1024], F32, tag="s", name="s")
                                   for _ in range(2)]
                            # scores: group by head so consecutive matmuls
                            # stay on one PSUM tile (no bank cycling)
                            for hh in range(2):
                                rows = slice(hh * 64, hh * 64 + 64)
                                for j, kt in enumerate(kts):
                                    k0, k1 = kt * 128, (kt + 1) * 128
                                    nc.tensor.matmul(
                                        ps2[hh][:, j * 512:(j + 1) * 512],
                                        KT[g][rows, k0:k1],
                                        QT[g][rows, q0:q1],
                                        start=True, stop=True)
                            ee = [epool.tile([128, 1024], BF16, tag="e", name="e")
                                  for _ in range(2)]
                            diag = (kts[0] >= 4 * qt)
                            for hh in range(2):
                                if not diag:
                                    nc.scalar.activation(
                                        ee[hh][:], ps2[hh][:], EXP,
                                        scale=SCALE)
                                else:
                                    for j, kt in enumerate(kts):
                                        off = kt * 128 - qt * 512
                                        cb = j * 512
                                        if off > 0:
                                            nc.gpsimd.memset(
                                                ee[hh][:, cb:cb + off], 0.0)
                                        nc.scalar.activation(
                                            ee[hh][:, cb + off:cb + 512],
                                            ps2[hh][:, cb + off:cb + 512],
                                            EXP, scale=SCALE)
                                        nc.vector.tensor_mul(
                                            ee[hh][:, cb + off:cb + off + 128],
                                            ee[hh][:, cb + off:cb + off + 128],
                                            tri[:])
                            for hh in range(2):
                                for j, kt in enumerate(kts):
                                    nc.tensor.matmul(
                                        yy[hh][:],
                                        VP[kt][:, 2 * g + hh, :],
                                        ee[hh][:, j * 512:(j + 1) * 512],
                                        start=(kt == 0), stop=(kt == n_kt - 1))
                        for hh in range(2):
                            den = norm.tile([1, 512], F32, tag="den")
                            nc.vector.tensor_copy(den[:], yy[hh][DH:DH + 1, :])
                            rd = norm.tile([1, 512], F32, tag="rd")
                            nc.vector.reciprocal_approx_fast(rd[:], den[:])
                            rdb = norm.tile([DH, 512], F32, tag="rdb")
                            nc.gpsimd.partition_broadcast(rdb[:], rd[:])
                            yn = norm.tile([DH, 512], F32, tag="yn")
                            nc.vector.tensor_mul(yn[:], yy[hh][0:DH, :], rdb[:])
                            nc.sync.dma_start(
                                out[2 * g + hh, :, q0:q1], yn[:])
    nc.compile()
    return nc


def get_nc():
    if "nc" not in _NC_CACHE:
        _NC_CACHE["nc"] = build_nc()
    return _NC_CACHE["nc"]


def make_in_maps(x, W_q, W_k, W_v):
    in_maps = []
    for core in range(N_CORES):
        b, hg = core // 2, core % 2
        rows = slice(hg * D_LOC, (hg + 1) * D_LOC)
        bf = ml_dtypes.bfloat16
        in_maps.append({
            "xb": np.ascontiguousarray(np.asarray(x[b], dtype=np.float32).astype(bf)),
            "wqb": np.ascontiguousarray(np.asarray(W_q[rows], dtype=np.float32).astype(bf)),
            "wkb": np.ascontiguousarray(np.asarray(W_k[rows], dtype=np.float32).astype(bf)),
            "wvb": np.ascontiguousarray(np.asarray(W_v[rows], dtype=np.float32).astype(bf)),
        })
    return in_maps


def assemble(results):
    Y = np.empty((B, H, S, DH), dtype=np.float32)
    for core in range(N_CORES):
        b, hg = core // 2, core % 2
        yc = results[core]["out"]  # [H_LOC, DH, S]
        Y[b, hg * H_LOC:(hg + 1) * H_LOC] = yc.transpose(0, 2, 1)
    return Y


def kernel(x, W_q, W_k, W_v):
    nc = get_nc()
    in_maps = make_in_maps(x, W_q, W_k, W_v)
    res = run_bass_kernel_spmd(nc, in_maps, list(range(N_CORES)))
    return assemble(res.results)
